# revision 1
# baseline (speedup 1.0000x reference)
"""Multi-head self-attention (b=8, t=2048, d=512, 8 heads x dk=64) on 8
Trainium2 NeuronCores.

Sharding: data-parallel over batch -- one batch element per core, no
collectives. Host slices inputs per core and stacks per-core outputs.

Per-core kernel (all matmuls bf16 -> fp32 PSUM):
  - Keys/values are computed only for UNMASKED key positions: the host
    gathers x rows where mask==1 into `xkv` (padded to a multiple of 128).
    Masked keys contribute exactly 0 post-softmax in the reference, so
    dropping them is mathematically identical and roughly halves both the
    score/ctx matmul work and the softmax-exp work. Padding rows are killed
    with a -1e30 additive bias folded into the exp activation.
  - x and xkv are pre-transposed on the HOST, so DMA lands directly in the
    feature-major layout every matmul wants -- no on-chip transposes.
  - Scores are computed transposed (S^T = [kv, q]) so softmax exp runs
    straight out of PSUM and P^T feeds the ctx matmul with no transposes.
  - V' carries a ones-column per head, so the softmax denominator falls out
    of the ctx matmul as row 64 (M = 65). The denominator row is
    reciprocal'd IN PLACE on partition 64 and broadcast to the 64 ctx
    partitions with a K=1 ones-column matmul (no cross-head gather).
  - ctx^T [dv, q] per head stacks directly into the feature-major activation
    layout the output projection needs.
  - v/out biases are added on DVE via host-replicated [128, 512] bias tiles
    (frees the PE from rank-1 bias matmuls).
  - Schedule: minimal prefix (first kT chunk + first qT halves) so the
    first exp lands ~13us in; every other projection/normalize/output step
    is a filler drained one-per-(kvi,half) slot inside the attention
    loops. The ctx matmuls ride a rolling deque ~3 half-slots behind
    their exp (crossing head-pair boundaries) so the PE never stalls
    in-order on the Activation engine. Weight/const DMAs ride the Pool
    queue (SWDGE) so the body x/xkv DMAs start immediately on the SP
    queue; the q+k weight columns go first on SP since the first matmuls
    need them.
"""

import sys
from contextlib import ExitStack

if "/opt/trn_rl_repo" not in sys.path:
    sys.path.insert(0, "/opt/trn_rl_repo")

import numpy as np
import ml_dtypes

import concourse.bass as bass
import concourse.mybir as mybir
import concourse.tile as tile

BF16 = ml_dtypes.bfloat16
T, D = 2048, 512
NH, DK = 8, 64
N_CORES = 8
NEG_BIG = -1.0e30

f32 = mybir.dt.float32
bf16 = mybir.dt.bfloat16


MAX_WAITS = 1

# scheduling knobs (sim-tuned)
PSUM_TAG_MODE = "shared"   # "shared" or "parity"
CTX_DEPTH = 6              # rolling ctx pipeline depth (slots)
WIDE_MM = False            # single N=1024 scores/ctx matmuls


def _split_excess_waits(nc, max_waits=MAX_WAITS):
    """Walrus in this container rejects instructions carrying more than
    ~2 sem-waits. Move the excess onto same-engine nops inserted just before
    the overloaded instruction (engine program order makes this equivalent:
    the engine blocks until every wait is observed either way)."""
    for f in nc.m.functions:
        for bb in f.blocks:
            out = []
            for inst in bb.instructions:
                si = getattr(inst, "sync_info", None)
                if si is not None and si.on_wait and len(si.on_wait) > max_waits:
                    waits = list(si.on_wait)
                    excess, keep = waits[:-max_waits], waits[-max_waits:]
                    si.on_wait = keep
                    for group in range(0, len(excess), max_waits):
                        nop = mybir.InstNoOp(
                            name=f"I-waitsplit-{nc.next_id()}",
                            engine=inst.engine,
                            ins=[],
                            outs=[],
                            sync_info=mybir.SyncInfo(
                                on_wait=excess[group : group + max_waits],
                                on_update=[],
                            ),
                        )
                        out.append(nop)
                out.append(inst)
            bb.instructions[:] = out


def _kv_chunks(total, step=512):
    chunks = []
    off = 0
    while off < total:
        c = min(step, total - off)
        chunks.append((off, c))
        off += c
    return chunks


def build_nc(t_kv: int, n_iters: int = 1) -> bass.Bass:
    """Build the per-core kernel. t_kv = padded gathered-key count (mult of
    128). n_iters > 1 repeats the whole body for timing."""
    nkv = t_kv // 128
    nc = bass.Bass()

    xT_h = nc.declare_dram_parameter("xT", [D, T], bf16, isOutput=False)
    xkvT_h = nc.declare_dram_parameter("xkvT", [D, t_kv], bf16, isOutput=False)
    biasm_h = nc.declare_dram_parameter("bias_m", [128, nkv], f32, isOutput=False)
    wqkv_h = nc.declare_dram_parameter("wqkv", [D, 3 * D], bf16, isOutput=False)
    bq_h = nc.declare_dram_parameter("bq", [128, 4], f32, isOutput=False)
    bk_h = nc.declare_dram_parameter("bk", [128, 4], f32, isOutput=False)
    bvrep_h = nc.declare_dram_parameter("bvrep", [128, D], bf16, isOutput=False)
    wout_h = nc.declare_dram_parameter("wout", [D, D], bf16, isOutput=False)
    boutrep_h = nc.declare_dram_parameter("boutrep", [128, D], f32, isOutput=False)
    out_h = nc.declare_dram_parameter("out", [T, D], f32, isOutput=True)

    with tile.TileContext(nc) as tc, ExitStack() as ctx:
        cpool = ctx.enter_context(tc.tile_pool(name="const", bufs=1))

        # ones in every partition: the K=1 denominator-broadcast matmul
        # streams from partition 64, so lhsT must sit at base partition 64.
        ones64 = cpool.tile([128, 64], bf16, tag="ones64")
        nc.vector.memset(ones64[:], 1.0)
        # vp lives in cpool so its ones-columns (denominator trick) are
        # memset once per NEFF, not once per iteration.
        vp = cpool.tile([128, nkv * 520], bf16, tag="vp", name="vp")

        # The head is DMA-BANDWIDTH-bound (~3.2MB of critical bytes at
        # ~360GB/s), so the SP queue is ordered by first-use: k-columns,
        # then the first xkv chunk, then just head-pair-0's q-columns,
        # then the first x chunk; everything else follows. (The remaining
        # SP-queue body DMAs are emitted inside _body in the same spirit.)
        wqkv_all = cpool.tile([128, 4 * 3 * D], bf16, tag="wqkv", name="wqkv_all")
        wqkv_sb = [wqkv_all[:, k * 3 * D : (k + 1) * 3 * D] for k in range(4)]
        wq3 = wqkv_all[:].rearrange("p (k c) -> p k c", k=4)
        wh3 = wqkv_h[:, :].rearrange("(k p) c -> p k c", k=4)
        nc.sync.dma_start(out=wq3[:, :, 512:1024], in_=wh3[:, :, 512:1024])
        # Remaining const/weight DMAs ride the Pool queue, ordered by first
        # use, so they don't delay the body's SP-queue x/xkv DMAs.
        bias_sb = cpool.tile([128, nkv], f32, tag="biasm")
        nc.gpsimd.dma_start(out=bias_sb[:], in_=biasm_h[:])
        bk_sb = cpool.tile([128, 4], f32, tag="bk")
        nc.gpsimd.dma_start(out=bk_sb[:], in_=bk_h[:])
        # v columns, one wide DMA (vp tiles are early consumers)
        nc.gpsimd.dma_start(out=wq3[:, :, 1024:1536], in_=wh3[:, :, 1024:1536])
        bq_sb = cpool.tile([128, 4], f32, tag="bq")
        nc.gpsimd.dma_start(out=bq_sb[:], in_=bq_h[:])
        bvrep_sb = cpool.tile([128, D], bf16, tag="bvrep")
        nc.gpsimd.dma_start(out=bvrep_sb[:], in_=bvrep_h[:])
        wout_all = cpool.tile([128, 4 * D], bf16, tag="wout", name="wout_all")
        wout_sb = [wout_all[:, k * D : (k + 1) * D] for k in range(4)]
        nc.gpsimd.dma_start(
            out=wout_all[:].rearrange("p (k c) -> p k c", k=4),
            in_=wout_h[:, :].rearrange("(k p) c -> p k c", k=4),
        )
        boutrep_sb = cpool.tile([128, D], f32, tag="boutrep")
        nc.gpsimd.dma_start(out=boutrep_sb[:], in_=boutrep_h[:])
        # memset after the DMA issues so the Pool engine doesn't delay them
        nc.gpsimd.memset(vp[:], 1.0)

        locals_dict = dict(
            t_kv=t_kv, nkv=nkv, ones64=ones64, vp=vp,
            wqkv_sb=wqkv_sb, wout_sb=wout_sb, bq_sb=bq_sb, bk_sb=bk_sb,
            bvrep_sb=bvrep_sb, boutrep_sb=boutrep_sb, bias_sb=bias_sb,
            xT_h=xT_h, xkvT_h=xkvT_h, out_h=out_h, wq3=wq3, wh3=wh3,
        )

        # NOTE: dynamic For_i loops wedge the device on this exec path
        # (BSP dispatch does not support branching) -- unroll instead.
        for _ in range(n_iters):
            _body(ctx, tc, nc, locals_dict)

    return nc


def _body(ctx, tc, nc, env):
    from collections import deque

    t_kv, nkv = env["t_kv"], env["nkv"]
    ones64, vp = env["ones64"], env["vp"]
    wqkv_sb, wout_sb = env["wqkv_sb"], env["wout_sb"]
    bq_sb, bk_sb = env["bq_sb"], env["bk_sb"]
    bvrep_sb, boutrep_sb = env["bvrep_sb"], env["boutrep_sb"]
    bias_sb = env["bias_sb"]
    xT_h, xkvT_h, out_h = env["xT_h"], env["xkvT_h"], env["out_h"]

    Exp = mybir.ActivationFunctionType.Exp
    add_op = mybir.AluOpType.add
    mult_op = mybir.AluOpType.mult
    QCH = 1024
    NQC = T // QCH

    with ExitStack() as bctx:
        persist = bctx.enter_context(tc.tile_pool(name="persist", bufs=1))
        ctxu_pool = bctx.enter_context(tc.tile_pool(name="ctxup", bufs=2))
        ld = bctx.enter_context(tc.tile_pool(name="ld", bufs=6))
        # PSUM (8 banks): tag "s" [128,1024] f32 = 2 banks x 2 bufs, shared
        # by every transient psum tile; pctx0/pctx1 [65,1024] = 2 banks each.
        mm = bctx.enter_context(tc.tile_pool(name="mm", bufs=(3 if PSUM_TAG_MODE == "shared" else 1), space="PSUM"))
        pctx_pool = bctx.enter_context(tc.tile_pool(name="pctx", bufs=1, space="PSUM"))
        sbw = bctx.enter_context(tc.tile_pool(name="sbw", bufs=8))

        # Scores ps tiles alternate between two single-buffer tags (s0/s1)
        # driven by slot parity; transient psum tiles (pk/pq/pv/po/pbc) take
        # the OPPOSITE parity. This keeps consecutive ps allocations on
        # different buffers (a 2-slot cushion against the exp) no matter how
        # many fillers are interleaved -- the old shared-tag rotation let a
        # filler collapse the cushion to 1, serializing scores behind exp.
        tagstate = {"ps": 0, "j": 0}

        def ps_tag():
            if PSUM_TAG_MODE == "shared":
                return "s"
            t = f"s{tagstate['ps'] % 2}"
            tagstate["ps"] += 1
            tagstate["j"] = 0
            return t

        def tmp_tag():
            if PSUM_TAG_MODE == "shared":
                return "s"
            t = f"s{(tagstate['ps'] + 1 + tagstate['j']) % 2}"
            tagstate["j"] += 1
            return t

        xT_all = persist.tile([128, 4 * T], bf16, tag="xTa", name="xT_all")
        xT = [xT_all[:, k * T : (k + 1) * T] for k in range(4)]
        xkvT_all = persist.tile([128, 4 * t_kv], bf16, tag="xkvTa", name="xkvT_all")
        xkvT = [xkvT_all[:, k * t_kv : (k + 1) * t_kv] for k in range(4)]
        xT3 = xT_all[:].rearrange("p (k c) -> p k c", k=4)
        xTh3 = xT_h[:, :].rearrange("(k p) c -> p k c", k=4)
        xkv3 = xkvT_all[:].rearrange("p (k c) -> p k c", k=4)
        xkvh3 = xkvT_h[:, :].rearrange("(k p) c -> p k c", k=4)
        qT = [persist.tile([128, T], bf16, tag=f"qT{m}", name=f"qT{m}") for m in range(4)]
        kT = [persist.tile([128, t_kv], bf16, tag=f"kT{m}", name=f"kT{m}") for m in range(4)]
        ctxT = [persist.tile([128, T], bf16, tag=f"ctxT{m}", name=f"ctxT{m}") for m in range(4)]

        # Body input DMAs on the SP queue, bandwidth-ordered by first use:
        # first xkv chunk (kT chunk 0), head-pair-0 q-columns, first x chunk
        # (qT[0]), then the rest.
        wq3b = env["wq3"]
        wh3b = env["wh3"]
        nc.sync.dma_start(out=xkv3[:, :, 0:512], in_=xkvh3[:, :, 0:512])
        nc.sync.dma_start(out=wq3b[:, :, 0:128], in_=wh3b[:, :, 0:128])
        nc.sync.dma_start(out=xT3[:, :, 0:QCH], in_=xTh3[:, :, 0:QCH])
        nc.sync.dma_start(out=wq3b[:, :, 128:512], in_=wh3b[:, :, 128:512])
        if t_kv > 512:
            nc.sync.dma_start(out=xkv3[:, :, 512:t_kv], in_=xkvh3[:, :, 512:t_kv])
        nc.sync.dma_start(out=xT3[:, :, QCH:T], in_=xTh3[:, :, QCH:T])

        # The per-engine instruction streams execute IN ORDER; anything that
        # should fill PE while ACT grinds exps must be EMITTED between
        # attention iterations. Fillers are zero-arg emitters drained inside
        # the attention loops.
        fillers = deque()

        def drain_filler(n=1):
            for _ in range(n):
                if fillers:
                    fillers.popleft()()

        def flush_fillers():
            while fillers:
                fillers.popleft()()

        # ---- emit helpers ----
        def emit_kT_chunk(m, off, clen):
            pk = mm.tile([128, 512], f32, tag=tmp_tag(), name="pk")
            for k in range(4):
                nc.tensor.matmul(
                    pk[:, :clen],
                    wqkv_sb[k][:, 512 + m * 128 : 512 + (m + 1) * 128],
                    xkvT[k][:, off : off + clen],
                    start=(k == 0),
                    stop=(k == 3),
                )
            nc.vector.tensor_scalar(
                kT[m][:, off : off + clen],
                pk[:, :clen], bk_sb[:, m : m + 1], None, add_op,
            )

        def emit_qT_half(m, o):
            pq = mm.tile([128, 512], f32, tag=tmp_tag(), name="pq")
            for k in range(4):
                nc.tensor.matmul(
                    pq[:],
                    wqkv_sb[k][:, m * 128 : (m + 1) * 128],
                    xT[k][:, o : o + 512],
                    start=(k == 0),
                    stop=(k == 3),
                )
            nc.vector.tensor_scalar(
                qT[m][:, o : o + 512],
                pq[:], bq_sb[:, m : m + 1], None, add_op,
            )

        def emit_vp_tile(mt):
            pv = mm.tile([128, 512], f32, tag=tmp_tag(), name="pv")
            for k in range(4):
                nc.tensor.matmul(
                    pv[:],
                    xkvT[k][:, mt * 128 : (mt + 1) * 128],
                    wqkv_sb[k][:, 1024:1536],
                    start=(k == 0),
                    stop=(k == 3),
                )
            dst = vp[:, mt * 520 : (mt + 1) * 520]
            dst3 = dst.rearrange("p (h c) -> p h c", c=65)[:, :, 0:64]
            src3 = pv[:].rearrange("p (h c) -> p h c", c=64)
            bv3 = bvrep_sb[:].rearrange("p (h c) -> p h c", c=64)
            nc.vector.tensor_tensor(dst3, src3, bv3, add_op)

        ctxu_all = {}
        # Rolling ctx pipeline: each (kvi, half) slot's ctx matmuls are
        # emitted ~2 slots later (one per slot), crossing hp boundaries.
        # Entries: (emit_ctx, after_fn_or_None) -- after_fn runs right after
        # the entry (used for the hp's pctx->ctxu copies after its last ctx).
        pending_ctxs = deque()

        def pop_pending(n=1):
            for _ in range(n):
                if not pending_ctxs:
                    return
                em, after = pending_ctxs.popleft()
                em()
                if after is not None:
                    after()

        def finish_attn():
            pop_pending(len(pending_ctxs))

        def emit_attn_head(qc, hp, hh):
            # Single-head kv walk (9 slots): only ONE [65,1024] ctx
            # accumulator is alive at a time, which frees 2 PSUM banks for a
            # THIRD scores buffer -- widening the PE-ahead cushion that
            # otherwise serializes scores behind exp.
            q0 = qc * QCH
            h = 2 * hp + hh
            prow = slice(hh * 64, hh * 64 + 64)
            pctx = pctx_pool.tile([65, QCH], f32, tag="pctx0", name="pctx0")

            def make_ctx(kvi, pt):
                def emit():
                    for c in range(2):
                        nc.tensor.matmul(
                            pctx[:, c * 512 : (c + 1) * 512],
                            vp[:, kvi * 520 + h * 65 : kvi * 520 + (h + 1) * 65],
                            pt[:, c * 512 : (c + 1) * 512],
                            start=(kvi == 0),
                            stop=(kvi == nkv - 1),
                        )
                return emit

            def finisher(pctx=pctx, qc=qc, h=h):
                cu = ctxu_pool.tile(
                    [65, QCH], bf16, tag=f"ctxu{h}", name=f"ctxu{h}"
                )
                nc.vector.tensor_copy(cu[:], pctx[:])
                # reciprocal of the denominator row, in place (bf16)
                with nc.allow_low_precision("softmax denom recip bf16"):
                    nc.vector.reciprocal(cu[64:65, :], cu[64:65, :])
                ctxu_all[(qc, h)] = cu

            for kvi in range(nkv):
                ks = slice(kvi * 128, (kvi + 1) * 128)
                ps = mm.tile([128, QCH], f32, tag=ps_tag(), name="ps")
                for c in range(2):
                    nc.tensor.matmul(
                        ps[:, c * 512 : (c + 1) * 512],
                        kT[hp][prow, ks],
                        qT[hp][prow, q0 + c * 512 : q0 + (c + 1) * 512],
                        start=True, stop=True,
                    )
                pt = sbw.tile([128, QCH], bf16, tag="pt", name="pt")
                nc.scalar.activation(
                    pt[:], ps[:], Exp,
                    bias=bias_sb[:, kvi : kvi + 1], scale=0.125,
                )
                last = kvi == nkv - 1
                pending_ctxs.append(
                    (make_ctx(kvi, pt), finisher if last else None)
                )
                if len(pending_ctxs) > CTX_DEPTH:
                    pop_pending(1)
                drain_filler(1)

        def emit_norm_head(qc, h, c):
            # broadcast the reciprocal'd denominator row to 64 partitions
            # (K=1 matmul), then multiply (walrus rejects DVE divide)
            q0 = qc * QCH
            cu = ctxu_all[(qc, h)]
            pbc = mm.tile([64, 512], f32, tag=tmp_tag(), name="pbc")
            nc.tensor.matmul(
                pbc[:],
                ones64[64:65, :],
                cu[64:65, c * 512 : (c + 1) * 512],
                start=True, stop=True,
            )
            nc.vector.tensor_tensor(
                ctxT[h // 2][(h % 2) * 64 : (h % 2) * 64 + 64,
                             q0 + c * 512 : q0 + (c + 1) * 512],
                cu[0:64, c * 512 : (c + 1) * 512],
                pbc[:], mult_op,
            )

        def emit_out_chunk(qc, mq):
            q0 = qc * QCH
            po = mm.tile([128, 512], f32, tag=tmp_tag(), name="po")
            mlo = q0 + mq * 128
            for k in range(4):
                nc.tensor.matmul(
                    po[:],
                    ctxT[k][:, mlo : mlo + 128],
                    wout_sb[k][:],
                    start=(k == 0),
                    stop=(k == 3),
                )
            ot = ld.tile([128, 512], f32, tag="ot", name="ot")
            nc.vector.tensor_tensor(ot[:], po[:], boutrep_sb[:], add_op)
            nc.sync.dma_start(out=out_h[mlo : mlo + 128, :], in_=ot[:])

        # ---- prefix: just enough for attn(qc0, hp0, kvi=0), plus the
        # first kT chunk of every head pair -- their deps (k-columns +
        # first xkv chunk) land early, and the PE would otherwise idle
        # here waiting for the x DMAs that qT needs. This also unloads
        # the over-committed attn(0,0) filler budget.
        for m in range(4):
            emit_kT_chunk(m, 0, min(512, t_kv))
        emit_qT_half(0, 0)
        emit_qT_half(0, 512)

        # Single-head blocks: 16 blocks of nkv slots. Deps for head-pair hp
        # (kT[hp] tail chunks + qT[hp] halves) drain during earlier blocks;
        # a filler at queue position p is emitted by end of drain p. The
        # deferred ctxu copy of block B pops ~CTX_DEPTH slots into block
        # B+1, so fillers needing it must sit at queue positions >= 3 of
        # B+1's list.
        kv_cks = _kv_chunks(t_kv)
        dep_seq = []  # block (0,0,0): own kv-walk deps, ordered by deadline
        for mt in range(nkv):
            dep_seq.append((mt + 2, lambda mt=mt: emit_vp_tile(mt)))
        for off, clen in kv_cks[1:]:
            dep_seq.append((off // 128 - 1, lambda o=off, c=clen: emit_kT_chunk(0, o, c)))
        dep_seq.sort(key=lambda x: x[0])
        fillers.extend(em for _, em in dep_seq)

        def hp_dep_fillers(hp_next, qc):
            fl = []
            for off, clen in kv_cks[1:]:  # chunk 0 was emitted in the prefix
                fl.append(lambda o=off, c=clen: emit_kT_chunk(hp_next, o, c))
            fl.append(lambda: emit_qT_half(hp_next, qc * QCH))
            fl.append(lambda: emit_qT_half(hp_next, qc * QCH + 512))
            return fl

        block_fill = {
            (0, 0, 0): [],  # dep_seq already queued
            (0, 0, 1): hp_dep_fillers(1, 0),
            (0, 1, 0): hp_dep_fillers(2, 0),
            (0, 1, 1): hp_dep_fillers(3, 0),
            (0, 2, 0): [lambda: emit_qT_half(0, QCH), lambda: emit_qT_half(0, QCH + 512)],
            (0, 2, 1): [lambda: emit_qT_half(1, QCH), lambda: emit_qT_half(1, QCH + 512)],
            (0, 3, 0): [lambda h=h: emit_norm_head(0, h, 0) for h in range(4)],
            (0, 3, 1): [lambda: emit_norm_head(0, 0, 1), lambda: emit_norm_head(0, 1, 1)]
            + [lambda: emit_norm_head(0, 4, 0), lambda: emit_norm_head(0, 5, 0)],
            (1, 0, 0): [lambda: emit_qT_half(2, QCH), lambda: emit_qT_half(2, QCH + 512)]
            + [lambda: emit_norm_head(0, 2, 1), lambda: emit_norm_head(0, 6, 0)]
            + [lambda: emit_norm_head(0, 3, 1), lambda: emit_norm_head(0, 7, 0)],
            (1, 0, 1): [lambda mq=mq: emit_out_chunk(0, mq) for mq in range(4)],
            (1, 1, 0): [lambda: emit_qT_half(3, QCH), lambda: emit_qT_half(3, QCH + 512)]
            + [lambda: emit_norm_head(0, 4, 1), lambda: emit_norm_head(0, 5, 1)],
            (1, 1, 1): [lambda: emit_norm_head(0, 6, 1), lambda: emit_norm_head(0, 7, 1)]
            + [lambda: emit_out_chunk(0, 4), lambda: emit_out_chunk(0, 5)],
            (1, 2, 0): [lambda: emit_out_chunk(0, 6), lambda: emit_out_chunk(0, 7)]
            + [lambda: emit_norm_head(1, 0, 0), lambda: emit_norm_head(1, 1, 0)],
            (1, 2, 1): [lambda: emit_norm_head(1, 0, 1), lambda: emit_norm_head(1, 1, 1)]
            + [lambda: emit_norm_head(1, 2, 0), lambda: emit_norm_head(1, 3, 0)],
            (1, 3, 0): [lambda: emit_norm_head(1, 2, 1), lambda: emit_norm_head(1, 3, 1)]
            + [lambda: emit_norm_head(1, 4, 0), lambda: None, lambda: None,
               lambda: emit_norm_head(1, 5, 0)],
            # depth-4 deque: fin(1,6) pops at slot 3 -- pad so norm(1,6,*)
            # sit at queue positions >= 3 (the None is a no-op drain)
            (1, 3, 1): [lambda: emit_norm_head(1, 4, 1), lambda: emit_norm_head(1, 5, 1)]
            + [lambda: None, lambda: None, lambda: None]
            + [lambda: emit_norm_head(1, 6, 0), lambda: emit_norm_head(1, 6, 1)],
        }
        for qc in range(2):
            for hp in range(4):
                for hh in range(2):
                    fillers.extend(block_fill[(qc, hp, hh)])
                    emit_attn_head(qc, hp, hh)
                    flush_fillers()

        # ---- tail: only head 7's norms + qc1 out-proj remain ----
        finish_attn()
        emit_norm_head(1, 7, 0)
        emit_out_chunk(1, 0)
        emit_norm_head(1, 7, 1)
        for mq in range(1, 8):
            emit_out_chunk(1, mq)


_NC_CACHE: dict = {}


def _get_nc(t_kv: int, n_iters: int = 1, split_waits: bool = True) -> bass.Bass:
    """split_waits rewrites sync-waits for the HW compiler; CoreSim must see
    the unsplit module, so sim tests pass split_waits=False."""
    key = (t_kv, n_iters)
    if key not in _NC_CACHE:
        nc = build_nc(t_kv, n_iters)
        _NC_CACHE[key] = [nc, False]
    ent = _NC_CACHE[key]
    if split_waits and not ent[1]:
        _split_excess_waits(ent[0])
        ent[1] = True
    return ent[0]


def make_in_maps(x, mask, Wqkv, bqkv, Wout, bout, t_kv: int):
    nkv = t_kv // 128
    shared = {
        "wqkv": np.ascontiguousarray(Wqkv).astype(BF16),
        "wout": np.ascontiguousarray(Wout).astype(BF16),
        "bq": np.ascontiguousarray(bqkv[0:512].reshape(4, 128).T).astype(np.float32),
        "bk": np.ascontiguousarray(bqkv[512:1024].reshape(4, 128).T).astype(np.float32),
        "bvrep": np.ascontiguousarray(
            np.tile(bqkv[1024:1536].reshape(1, 512), (128, 1))
        ).astype(BF16),
        "boutrep": np.ascontiguousarray(
            np.tile(bout.reshape(1, 512), (128, 1))
        ).astype(np.float32),
    }
    x16 = np.asarray(x).astype(BF16)  # cast once, transpose 2-byte views
    in_maps = []
    for b in range(N_CORES):
        idx = np.nonzero(mask[b, 0] != 0)[0]
        cnt = len(idx)
        xkvT = np.zeros((D, t_kv), dtype=BF16)
        xkvT[:, :cnt] = x16[b][idx].T
        biasvec = np.where(np.arange(t_kv) < cnt, 0.0, NEG_BIG).astype(np.float32)
        bias_m = np.ascontiguousarray(biasvec.reshape(nkv, 128).T)
        in_maps.append({
            **shared,
            "xT": np.ascontiguousarray(x16[b].T),
            "xkvT": xkvT,
            "bias_m": bias_m,
        })
    return in_maps


def pick_t_kv(mask) -> int:
    counts = (np.asarray(mask)[:, 0, :] != 0).sum(axis=1)
    # Floor of 1024 (8 kv tiles): the block schedule's filler-position
    # invariants assume >= 8 slots per attention block. Padding rows are
    # killed by the -1e30 exp bias, so a larger t_kv is always correct.
    return max(1024, int(-(-int(counts.max()) // 128)) * 128)


def kernel(x, mask, Wqkv, bqkv, Wout, bout):
    from concourse.bass_utils import run_bass_kernel_spmd

    x = np.asarray(x, dtype=np.float32)
    mask = np.asarray(mask)
    Wqkv = np.asarray(Wqkv, dtype=np.float32)
    bqkv = np.asarray(bqkv, dtype=np.float32)
    Wout = np.asarray(Wout, dtype=np.float32)
    bout = np.asarray(bout, dtype=np.float32)

    t_kv = pick_t_kv(mask)
    nc = _get_nc(t_kv)
    in_maps = make_in_maps(x, mask, Wqkv, bqkv, Wout, bout, t_kv)
    res = run_bass_kernel_spmd(nc, in_maps, list(range(N_CORES)))
    out = np.stack([res.results[i]["out"] for i in range(N_CORES)], axis=0)
    return out.astype(np.float32)

